# revision 9
# baseline (speedup 1.0000x reference)
"""GCN2Conv (variant=False) Trainium2 kernel, v2.

Math (all linear, so theta folds out of the critical path):
  out = support @ T',              T' = beta*theta + (1-beta)*I
  support = c1*hi + alpha*h0,      c1 = 1-alpha
  hi = dis_r . ((A+I) @ (dis . x)),  dis = (rowsum(A)+1)^-1/2
=>
  out[R] = dis[R] . (A_R @ Gd) + dis[R]^2 . G[R] + H[R]
  G = c1 * (x @ T'),  Gd = dis . G,  H = alpha * (h0[R] @ T')

Sharding: B=4 graphs x 2 cores/graph. Core pair (2g, 2g+1) owns rows
[0:1536) / [1536:3000) of graph g (128-aligned halves; everything zero
padded to N_PAD=3072, M_PAD=1536 so the SPMD program is identical on
both halves).

Host passes pure layout transforms only (slice / transpose / pad / fp16
cast): AT = A[R,:].T as [n_chunks, KT, 128, CHUNK] fp16 so stream DMAs
are contiguous; xT_loc = x[R].T, h0T = h0[R].T fp16; x_full fp16;
theta fp32. Output is produced transposed [F, M_PAD] fp32 and the host
transposes it back while unsharding.

Device pipeline per core:
  - 3 sub-streams (one per 512-wide m-chunk of A^T), 4 DMAs each.
  - PE ones-vector matmuls reduce A^T over partitions per chunk =
    row degrees of own rows; +1 self loop on copy-out.
  - 3 pipelined pair-AllGathers (2KB each) exchange degree chunks; each
    unlocks a "wave" of 8 k-blocks (4 even-side + 4 odd-side).
  - Gd = dis . G per k-block; main matmul rawT[f,m] += Gd_kb^T AT_kb
    accumulates 24 k-blocks into 3 PSUM banks, emitted interleaved with
    the stream-chasing rowsums (PE executes in order).
  - Epilogue per chunk: outT = rawT . dis_R + (G_R/deg_R + H)^T; one
    768KB store of outT [128, 1536] fp32.
"""

import math
import sys

import numpy as np

sys.path.insert(0, "/opt/trn_rl_repo")

import concourse.bacc as bacc
import concourse.mybir as mybir
import concourse.tile as tile
from concourse import bass_utils, masks
from concourse.mybir import dt

AF = mybir.ActivationFunctionType

F = 128            # feature dim
P = 128            # SBUF partitions

B_FULL, N_FULL = 4, 3000
N_CORES_FULL = 8
M_PAD_FULL = 1536          # even core rows [0:1536), odd [1536:3000)
N_PAD_FULL = 3072
CHUNK_FULL = 512
NCH = 3                    # m-chunks / AG waves (schedule hardcoded for 3)


def build_program(n_pad, m_pad, chunk, n_cores, alpha, beta, n_quarters=4):
    """Build the SPMD Bass program (identical on every core).

    Per-core inputs:
      adjT  [nch*KT*128, chunk] f16 : A[R,:].T padded, chunk-major
      x_full [n_pad, F] f16, xT_loc [F, m_pad] f16, h0T [F, m_pad] f16,
      theta [F, F] f32.
    Output: outT [F, m_pad] f32 (transposed).
    """
    assert n_pad == 2 * m_pad and m_pad % chunk == 0 and chunk % P == 0
    KT = n_pad // P                 # k tiles (contraction blocks)
    nch = m_pad // chunk            # m-chunks == AG waves
    assert nch == NCH
    K = chunk // P                  # k-blocks unlocked per wave per side
    kb_odd = m_pad // P             # first odd-side k-block
    assert KT % n_quarters == 0
    kb_per_q = KT // n_quarters
    c1 = 1.0 - alpha

    # wave_c k-blocks: even rows [c*K, (c+1)*K) + odd rows shifted
    waves = [list(range(c * K, (c + 1) * K))
             + list(range(kb_odd + c * K, kb_odd + (c + 1) * K))
             for c in range(nch)]

    nc = bacc.Bacc(
        "TRN2", target_bir_lowering=False, debug=False, num_devices=n_cores
    )
    adjT = nc.dram_tensor(
        "adjT", [nch * KT * P, chunk], dt.float16, kind="ExternalInput"
    )
    x_full = nc.dram_tensor("x_full", [P, (n_pad // P) * F], dt.float16,
                            kind="ExternalInput")
    xT_loc = nc.dram_tensor("xT_loc", [F, m_pad], dt.float16, kind="ExternalInput")
    h0T = nc.dram_tensor("h0T", [F, m_pad], dt.float16, kind="ExternalInput")
    theta = nc.dram_tensor("theta", [F, F], dt.float32, kind="ExternalInput")
    outT_d = nc.dram_tensor("outT", [F, m_pad], dt.float32, kind="ExternalOutput")

    groups = [[2 * g, 2 * g + 1] for g in range(max(1, n_cores // 2))]

    with tile.TileContext(nc) as tc:
        from contextlib import ExitStack

        with ExitStack() as ctx:
            ep = ctx.enter_context

            consts = ep(tc.tile_pool(name="consts", bufs=1))
            at_pool = ep(tc.tile_pool(name="at", bufs=1))
            xs_pool = ep(tc.tile_pool(name="xs", bufs=1))
            deg_pool = ep(tc.tile_pool(name="deg", bufs=1))
            out_pool = ep(tc.tile_pool(name="out", bufs=1))
            ps_raw = ep(tc.tile_pool(name="ps_raw", bufs=3, space="PSUM"))
            ps_deg = ep(tc.tile_pool(name="ps_deg", bufs=2, space="PSUM"))
            ps_sm = ep(tc.tile_pool(name="ps_sm", bufs=2, space="PSUM"))
            dram = ep(tc.tile_pool(name="dram", bufs=1, space="DRAM"))

            # -- kick the collective-stream init as early as possible --
            cc_warm_in = dram.tile([P], dt.float32, name="cc_warm_in")
            cc_warm_out = dram.tile([2 * P], dt.float32, name="cc_warm_out")
            nc.gpsimd.collective_compute(
                "AllGather",
                mybir.AluOpType.bypass,
                replica_groups=groups,
                ins=[cc_warm_in[:]],
                outs=[cc_warm_out[:]],
            )

            # ---------------- constants -----------------------------------
            ident = consts.tile([P, P], dt.float32)
            masks.make_identity(nc, ident[:])
            ident16 = consts.tile([P, P], dt.float16)
            nc.vector.tensor_copy(ident16[:], ident[:])
            ones = consts.tile([P, 1], dt.float16)
            nc.gpsimd.memset(ones[:], 1.0)

            theta_sb = consts.tile([F, F], dt.float32)
            nc.scalar.dma_start(theta_sb[:], theta[:])
            # T' = beta*theta + (1-beta)*I ; thG = c1*T' ; thH = alpha*T'
            thetaP = consts.tile([F, F], dt.float32)
            nc.vector.tensor_scalar_mul(thetaP[:], theta_sb[:], beta)
            nc.vector.scalar_tensor_tensor(
                thetaP[:], ident[:], 1.0 - beta, thetaP[:],
                mybir.AluOpType.mult, mybir.AluOpType.add,
            )
            thG = consts.tile([F, F], dt.float16)
            nc.vector.tensor_scalar_mul(thG[:], thetaP[:], c1)
            thH = consts.tile([F, F], dt.float16)
            nc.vector.tensor_scalar_mul(thH[:], thetaP[:], alpha)

            # ---------------- bulk loads -----------------------------------
            # A^T resident: free layout (chunk, kb, m) fp16
            AT = at_pool.tile([P, nch * KT * chunk], dt.float16)
            AT4 = AT[:].rearrange("p (c kb m) -> p c kb m", c=nch, kb=KT)
            adjT_ap = adjT[:].rearrange(
                "(c p kb) m -> p c kb m", c=nch, p=P, kb=KT
            )

            def emit_stream(c, h):
                k0 = h * (KT // 2)
                nc.sync.dma_start(
                    AT4[:, c, k0 : k0 + KT // 2, :],
                    adjT_ap[:, c, k0 : k0 + KT // 2, :],
                )

            for c in range(nch):
                for h in range(2):
                    emit_stream(c, h)

            # x in per-k-tile layout [p, kb, f]
            xg = xs_pool.tile([P, KT * F], dt.float16)
            nc.scalar.dma_start(xg[:], x_full[:])
            xTl = xs_pool.tile([P, m_pad], dt.float16, tag="xTl")
            nc.scalar.dma_start(xTl[:], xT_loc[:])
            h0T_sb = xs_pool.tile([P, m_pad], dt.float16, tag="h0T")
            nc.scalar.dma_start(h0T_sb[:], h0T[:])

            # ---------------- degree rowsums (PE partition-reduce) ---------
            degrow = deg_pool.tile([1, m_pad], dt.float32)
            rcp = deg_pool.tile([1, m_pad], dt.float32, tag="rcp")
            rs_row = deg_pool.tile([1, m_pad], dt.float32, tag="rs_row")
            rs_b = deg_pool.tile([P, m_pad], dt.float32, tag="rs_b")
            deg_ps_tiles = {}

            def emit_rowsums(c, q):
                if q == 0:
                    deg_ps_tiles[c] = ps_deg.tile(
                        [1, chunk], dt.float32, name=f"deg_ps_{c}",
                        tag="degps", bufs=2,
                    )
                dps = deg_ps_tiles[c]
                for kb in range(q * kb_per_q, (q + 1) * kb_per_q):
                    nc.tensor.matmul(
                        dps[0:1, :], ones[:, 0:1], AT4[:, c, kb, :],
                        start=(kb == 0), stop=(kb == KT - 1),
                    )

            def emit_deg_chunk_post(c):
                # +1 self loop on psum -> sbuf copy; local dis pieces
                s = c * chunk
                dps = deg_ps_tiles[c]
                nc.vector.tensor_scalar_add(
                    degrow[0:1, s : s + chunk], dps[0:1, :], 1.0
                )
                nc.vector.reciprocal(
                    rcp[0:1, s : s + chunk], degrow[0:1, s : s + chunk]
                )
                nc.scalar.sqrt(
                    rs_row[0:1, s : s + chunk], rcp[0:1, s : s + chunk]
                )
                nc.gpsimd.partition_broadcast(
                    rs_b[:, s : s + chunk], rs_row[0:1, s : s + chunk]
                )

            # ---------------- x^T transposes + G ---------------------------
            xT = xs_pool.tile([P, KT * F], dt.float16, tag="xT")
            G = xs_pool.tile([P, KT * F], dt.float16, tag="G")
            Gd = xs_pool.tile([P, KT * F], dt.float16, tag="Gd")

            def emit_xt_g(kb):
                tp = ps_sm.tile([P, P], dt.float16, tag="sm")
                nc.tensor.transpose(
                    tp[:P, :P], xg[:, kb * F : (kb + 1) * F], ident16[:P, :P]
                )
                nc.scalar.activation(
                    xT[:, kb * F : (kb + 1) * F], tp[:P, :P], AF.Copy
                )
                gp = ps_sm.tile([P, F], dt.float32, tag="sm")
                nc.tensor.matmul(
                    gp[:P, :F], xT[:, kb * F : (kb + 1) * F], thG[:, :],
                    start=True, stop=True,
                )
                nc.scalar.activation(
                    G[:, kb * F : (kb + 1) * F], gp[:P, :F], AF.Copy
                )

            # ---------------- QT = (G_R / deg_R + H)^T ---------------------
            QT = out_pool.tile([P, m_pad], dt.float32, tag="QT")
            GoT = out_pool.tile([P, m_pad], dt.float16, tag="GoT")
            rs2_b = deg_pool.tile([P, chunk], dt.float32, tag="rs2_b")

            def emit_goh(c):
                s = c * chunk
                hp = ps_sm.tile([P, chunk], dt.float32, tag="sm")
                nc.tensor.matmul(
                    hp[:F, :chunk], thH[:, :], h0T_sb[:, s : s + chunk],
                    start=True, stop=True,
                )
                nc.scalar.activation(QT[:, s : s + chunk], hp[:F, :chunk], AF.Copy)
                gp2 = ps_sm.tile([P, chunk], dt.float32, tag="sm")
                nc.tensor.matmul(
                    gp2[:F, :chunk], thG[:, :], xTl[:, s : s + chunk],
                    start=True, stop=True,
                )
                nc.scalar.activation(GoT[:, s : s + chunk], gp2[:F, :chunk], AF.Copy)

            def emit_qt(c):
                # QT += GoT * (1/deg) broadcast
                s = c * chunk
                nc.gpsimd.partition_broadcast(rs2_b[:, :], rcp[0:1, s : s + chunk])
                tmp = deg_pool.tile([P, chunk], dt.float32, tag="qtmp")
                nc.vector.tensor_mul(tmp[:, :], GoT[:, s : s + chunk], rs2_b[:, :])
                nc.vector.tensor_add(
                    QT[:, s : s + chunk], QT[:, s : s + chunk], tmp[:, :]
                )

            # ---------------- degree exchange (pipelined AGs) --------------
            deg_loc_d = dram.tile([m_pad], dt.float32)
            deg_pair_d = [dram.tile([2 * chunk], dt.float32, tag=f"dp{c}",
                                    name=f"deg_pair_{c}")
                          for c in range(nch)]
            disg = deg_pool.tile([P, KT], dt.float32, tag="disg")

            def emit_ag(c):
                s = c * chunk
                nc.scalar.dma_start(
                    deg_loc_d[s : s + chunk].rearrange("(a m) -> a m", a=1),
                    degrow[0:1, s : s + chunk],
                )
                nc.gpsimd.collective_compute(
                    "AllGather",
                    mybir.AluOpType.bypass,
                    replica_groups=groups,
                    ins=[deg_loc_d[s : s + chunk]],
                    outs=[deg_pair_d[c][:]],
                )

            def emit_wave_dis(c):
                # AG output: [even chunk degs | odd chunk degs] -> disg cols
                for side in range(2):
                    dg = deg_pool.tile([K, P], dt.float32, tag="dgT", bufs=2)
                    nc.scalar.dma_start(
                        dg[:, :],
                        deg_pair_d[c][side * chunk : (side + 1) * chunk]
                        .rearrange("(a b) -> a b", b=P),
                    )
                    tp = ps_sm.tile([P, K], dt.float32, tag="sm")
                    nc.tensor.transpose(tp[:P, :K], dg[:K, :P], ident[:K, :K])
                    kb0 = side * kb_odd + c * K
                    nc.vector.reciprocal(disg[:, kb0 : kb0 + K], tp[:P, :K])
                    nc.scalar.sqrt(disg[:, kb0 : kb0 + K], disg[:, kb0 : kb0 + K])

            def emit_gd(kbs):
                for kb in kbs:
                    nc.vector.tensor_scalar_mul(
                        Gd[:, kb * F : (kb + 1) * F],
                        G[:, kb * F : (kb + 1) * F],
                        disg[:, kb : kb + 1],
                    )

            # ---------------- main matmul + epilogue ------------------------
            raw_ps = [ps_raw.tile([P, chunk], dt.float32, name=f"raw_{c}",
                                  tag=f"raw{c}", bufs=1)
                      for c in range(nch)]
            n_mm_done = [0] * nch
            outT_sb = out_pool.tile([P, m_pad], dt.float32, tag="outT")

            def emit_mm(kbs, c):
                for kb in kbs:
                    nc.tensor.matmul(
                        raw_ps[c][:F, :chunk],
                        Gd[:, kb * F : (kb + 1) * F],
                        AT4[:, c, kb, :],
                        start=(n_mm_done[c] == 0),
                        stop=(n_mm_done[c] == KT - 1),
                    )
                    n_mm_done[c] += 1

            def emit_epilogue(c):
                s = c * chunk
                nc.vector.tensor_mul(
                    outT_sb[:, s : s + chunk], raw_ps[c][:F, :chunk],
                    rs_b[:, s : s + chunk],
                )
                nc.vector.tensor_add(
                    outT_sb[:, s : s + chunk], outT_sb[:, s : s + chunk],
                    QT[:, s : s + chunk],
                )

            # ---------------- emission schedule (PE is in-order) ------------
            # quarter q of chunk c lands ~ (4c+q+1)*2.2us; AG_c ~ stream of
            # chunk c + rowsums + AG latency. Interleave so PE never blocks
            # on a not-yet-landed quarter while ready work exists.
            for kb in range(KT):
                emit_xt_g(kb)
            for c in range(nch):
                emit_goh(c)
            for q in range(n_quarters):
                emit_rowsums(0, q)
            emit_deg_chunk_post(0)
            emit_qt(0)
            emit_ag(0)
            emit_rowsums(1, 0)
            emit_rowsums(1, 1)
            emit_wave_dis(0)
            emit_gd(waves[0])
            emit_mm(waves[0], 0)
            emit_rowsums(1, 2)
            emit_rowsums(1, 3)
            emit_deg_chunk_post(1)
            emit_qt(1)
            emit_ag(1)
            emit_mm(waves[0], 1)
            emit_rowsums(2, 0)
            emit_wave_dis(1)
            emit_gd(waves[1])
            emit_mm(waves[1], 0)
            emit_rowsums(2, 1)
            emit_mm(waves[1], 1)
            emit_rowsums(2, 2)
            # chunk-2 MMs for already-unlocked kbs, grouped by landed quarter
            ready01 = sorted(waves[0] + waves[1])
            emit_mm([kb for kb in ready01 if kb < 3 * kb_per_q], 2)
            emit_rowsums(2, 3)
            emit_deg_chunk_post(2)
            emit_qt(2)
            emit_ag(2)
            emit_mm([kb for kb in ready01 if kb >= 3 * kb_per_q], 2)
            emit_wave_dis(2)
            emit_gd(waves[2])
            emit_mm(waves[2], 0)
            emit_epilogue(0)
            emit_mm(waves[2], 1)
            emit_epilogue(1)
            emit_mm(waves[2], 2)
            emit_epilogue(2)
            nc.sync.dma_start(outT_d[:], outT_sb[:])

    nc.compile()
    return nc


def make_in_maps(x, adj, h0, theta, n_cores, n_pad, m_pad, chunk, n_real):
    KT = n_pad // P
    nch = m_pad // chunk
    f2 = np.float16
    in_maps = []
    x_pads = {}
    for c in range(n_cores):
        g, h = c // 2, c % 2
        r0 = 0 if h == 0 else m_pad
        m_real = m_pad if h == 0 else n_real - m_pad
        if g not in x_pads:
            xp = np.zeros((n_pad, F), f2)
            xp[:n_real] = x[g].astype(f2)
            x_pads[g] = np.ascontiguousarray(
                xp.reshape(KT, P, F).transpose(1, 0, 2)
            ).reshape(P, KT * F)
        at = np.zeros((n_pad, m_pad), f2)
        at[:n_real, :m_real] = adj[g, r0 : r0 + m_real, :].astype(f2).T
        at = np.ascontiguousarray(
            at.reshape(KT, P, nch, chunk).transpose(2, 1, 0, 3)
        ).reshape(nch * P * KT, chunk)
        xt = np.zeros((F, m_pad), f2)
        xt[:, :m_real] = x[g, r0 : r0 + m_real, :].astype(f2).T
        ht = np.zeros((F, m_pad), f2)
        ht[:, :m_real] = h0[g, r0 : r0 + m_real, :].astype(f2).T
        in_maps.append(
            {
                "adjT": at,
                "x_full": x_pads[g],
                "xT_loc": xt,
                "h0T": ht,
                "theta": theta.astype(np.float32),
            }
        )
    return in_maps


_CACHE = {}


def _get_program(key, *args, **kwargs):
    if key not in _CACHE:
        _CACHE[key] = build_program(*args, **kwargs)
    return _CACHE[key]


def kernel(x, adj, h0, theta, lamda, alpha, l):
    x = np.asarray(x, dtype=np.float32)
    adj = np.asarray(adj, dtype=np.float32)
    h0 = np.asarray(h0, dtype=np.float32)
    theta = np.asarray(theta, dtype=np.float32)
    lamda_f = float(np.asarray(lamda))
    alpha_f = float(np.asarray(alpha))
    l_f = float(np.asarray(l))
    beta_f = float(math.log(lamda_f / l_f + 1.0))

    B, N, Fdim = x.shape
    assert (B, N, Fdim) == (B_FULL, N_FULL, F)

    nc = _get_program(
        ("full", alpha_f, beta_f),
        N_PAD_FULL, M_PAD_FULL, CHUNK_FULL, N_CORES_FULL, alpha_f, beta_f,
    )

    in_maps = make_in_maps(
        x, adj, h0, theta, N_CORES_FULL,
        N_PAD_FULL, M_PAD_FULL, CHUNK_FULL, N_FULL,
    )
    res = bass_utils.run_bass_kernel_spmd(
        nc, in_maps, list(range(N_CORES_FULL))
    ).results

    out = np.empty((B, N, Fdim), dtype=np.float32)
    for c in range(N_CORES_FULL):
        g, h = c // 2, c % 2
        r0 = 0 if h == 0 else M_PAD_FULL
        m_real = M_PAD_FULL if h == 0 else N - M_PAD_FULL
        out[g, r0 : r0 + m_real, :] = res[c]["outT"][:, :m_real].T
    return out
